# revision 7
# baseline (speedup 1.0000x reference)
"""Trainium2 Bass kernel for fused MHA (QKV proj + RoPE + causal attention + out proj).

Sharding: 8 cores = 4 batches x 2 query-halves. Each core handles one batch's
full K/V (redundant across the pair) and a balanced set of 1024 query rows
(global 256-row blocks {0,3,4,7} or {1,2,5,6}), so causal work is balanced and
all cores run one SPMD program; per-core differences are pure input data
(gathered x columns, RoPE tables at the gathered positions, causal mask tiles).
Host gathers the per-core outputs back into the full [4, 2048, 1024] tensor.

Device algorithm (everything transposed so softmax reduces along PSUM
partitions for free):
  Qb^T = Wq @ xq^T + b      (bf16, PE)
  Qs^T = Pswap @ Qb^T       (pair swap as a matmul; RoPE = Qb*cos + Qs*sin)
  S^T[k,q] = Krot^T(head)^T-slices @ Qrot -> psum, exp(0.125*S) on ACT
  P masked by data-driven causal tiles; U[65,q] += Vaug[k,65]^T P (ones col
  of Vaug accumulates the softmax denominator l)
  O^T = U[0:64] * (1/l broadcast via a K=1 outer-product matmul)
  out[q,e] = sum_heads O^T(head)^T @ Wo^T + b_o
"""

import os
import sys

import numpy as np
import ml_dtypes

for _p in ("/opt/trn_rl_repo",):
    if os.path.isdir(_p) and _p not in sys.path:
        sys.path.insert(0, _p)

import concourse.bass as bass  # noqa: E402
import concourse.tile as tile  # noqa: E402
import concourse.mybir as mybir  # noqa: E402
from concourse import bacc  # noqa: E402

BF16 = mybir.dt.bfloat16
F16 = mybir.dt.float16
F32 = mybir.dt.float32
AF = mybir.ActivationFunctionType
BFNP = ml_dtypes.bfloat16

S = 2048        # sequence length
D = 1024        # model dim
NH = 16         # heads
DK = 64         # head dim
NQ = 1024       # queries per core
QB = 256        # query block (matmul N for attention)
KC = 128        # k chunk (partition tile)
CCH = (4, 8, 12, 16)   # k-chunks computed per local q-block (uniform SPMD)
GBLK = ((0, 3, 4, 7), (1, 2, 5, 6))  # global 256-row blocks by core parity


def emit(tc: tile.TileContext, io: dict, ctx):
    nc = tc.nc
    from contextlib import ExitStack

    # Pools are stack-allocated: phase-1 pools (x, tables, W-staging) are
    # released before phase-2 pools (masks, Ocat, attention temps) allocate,
    # so their SBUF is reclaimed.
    persist = ctx.enter_context(tc.tile_pool(name="persist", bufs=1))
    ppsum = ctx.enter_context(tc.tile_pool(name="ppsum", bufs=2, space="PSUM"))

    def bcast_rows(dram_ap):
        # [n] dram vector -> [128, n] partition-broadcast AP
        return bass.AP(
            tensor=dram_ap.tensor,
            offset=dram_ap.offset,
            ap=[[0, 128]] + list(dram_ap.ap),
        )

    # ---- persistent tiles (live across both phases) ----
    bQK = persist.tile([128, 16], F32, name="bQK_sb")
    nc.sync.dma_start(out=bQK, in_=io["bQK"])
    ones_t = persist.tile([65, 64], F32, name="ones_sb")
    nc.vector.memset(ones_t[:, :], 1.0)

    Qrot = [persist.tile([128, NQ], BF16, name=f"qrot{r}") for r in range(8)]
    Krot = [persist.tile([128, S], BF16, name=f"krot{r}") for r in range(8)]
    Vaug = [persist.tile([128, NH, 65], BF16, name=f"vaug{c}") for c in range(16)]

    for c in range(16):
        nc.vector.memset(Vaug[c][:, :, 64:65], 1.0)

    # ================= phase 1: projections + RoPE =================
    with ExitStack() as ph1:
        p1 = ph1.enter_context(tc.tile_pool(name="p1", bufs=1))
        wpool = ph1.enter_context(tc.tile_pool(name="wpool", bufs=2))
        qkb = ph1.enter_context(tc.tile_pool(name="qkb", bufs=4))
        tmp = ph1.enter_context(tc.tile_pool(name="tmp", bufs=3))
        spsum = ph1.enter_context(tc.tile_pool(name="spsum", bufs=2, space="PSUM"))

        xT = p1.tile([128, 8, S], BF16, name="xT_sb")
        nc.sync.dma_start(out=xT, in_=io["xT"])
        xqT = p1.tile([128, 8, NQ], BF16, name="xqT_sb")
        nc.sync.dma_start(out=xqT, in_=io["xqT"])
        cosK = p1.tile([128, S], F16, name="cosK_sb")
        nc.sync.dma_start(out=cosK, in_=io["cosK"])
        sinK = p1.tile([128, S], F16, name="sinK_sb")
        nc.sync.dma_start(out=sinK, in_=io["sinK"])
        cosQ = p1.tile([128, NQ], F16, name="cosQ_sb")
        nc.sync.dma_start(out=cosQ, in_=io["cosQ"])
        sinQ = p1.tile([128, NQ], F16, name="sinQ_sb")
        nc.sync.dma_start(out=sinQ, in_=io["sinQ"])
        pswap = p1.tile([128, 128], BF16, name="pswap_sb")
        nc.sync.dma_start(out=pswap, in_=io["pswap"])
        bVt = p1.tile([128, D], F32, name="bV_sb")
        nc.sync.dma_start(out=bVt, in_=bcast_rows(io["bV"]))

        def proj_rope(wcol0, rhs, n_sg, cos_t, sin_t, dest, bias0, wname):
            # dest[rt][:, sg] = RoPE( W[:, wcol0+...]^T-slices . rhs + bias )
            for half in range(2):
                wt = wpool.tile([128, 8, 512], BF16, name=f"{wname}{half}", tag="wph")
                nc.sync.dma_start(
                    out=wt,
                    in_=io["wqkvT"][
                        :, :, wcol0 + 512 * half : wcol0 + 512 * (half + 1)
                    ],
                )
                for rtl in range(4):
                    rt = 4 * half + rtl
                    for sg in range(n_sg):
                        ps = ppsum.tile([128, 512], F32, name="projps", tag="projps")
                        for ic in range(8):
                            nc.tensor.matmul(
                                ps,
                                lhsT=wt[:, ic, 128 * rtl : 128 * (rtl + 1)],
                                rhs=rhs[:, ic, 512 * sg : 512 * (sg + 1)],
                                start=(ic == 0),
                                stop=(ic == 7),
                            )
                        qb = qkb.tile([128, 512], BF16, name="qkbt", tag="qkbt")
                        nc.vector.tensor_scalar_add(
                            qb, ps, bQK[:, bias0 + rt : bias0 + rt + 1]
                        )
                        sps = spsum.tile([128, 512], F32, name="swapps", tag="swapps")
                        nc.tensor.matmul(sps, lhsT=pswap, rhs=qb, start=True, stop=True)
                        nc.vector.tensor_mul(
                            sps, sps, sin_t[:, 512 * sg : 512 * (sg + 1)]
                        )
                        t1 = tmp.tile([128, 512], F32, name="ropet", tag="ropet")
                        nc.vector.tensor_mul(
                            t1, qb, cos_t[:, 512 * sg : 512 * (sg + 1)]
                        )
                        nc.vector.tensor_add(
                            dest[rt][:, 512 * sg : 512 * (sg + 1)], t1, sps
                        )

        proj_rope(0, xqT, 2, cosQ, sinQ, Qrot, 0, "wq")
        proj_rope(D, xT, 4, cosK, sinK, Krot, 8, "wk")

        # ---- V projection (natural [s, vc] layout) ----
        for vg in range(2):
            wv = wpool.tile([128, 8, 512], BF16, name=f"wv{vg}", tag="wph")
            nc.sync.dma_start(
                out=wv,
                in_=io["wqkvT"][:, :, 2 * D + 512 * vg : 2 * D + 512 * (vg + 1)],
            )
            for st in range(16):
                ps = ppsum.tile([128, 512], F32, name="vps", tag="projps")
                for ic in range(8):
                    nc.tensor.matmul(
                        ps,
                        lhsT=xT[:, ic, 128 * st : 128 * (st + 1)],
                        rhs=wv[:, ic, :],
                        start=(ic == 0),
                        stop=(ic == 7),
                    )
                nc.vector.tensor_add(
                    Vaug[st][:, 8 * vg : 8 * (vg + 1), 0:64],
                    ps.rearrange("p (h d) -> p h d", d=64),
                    bVt[:, 512 * vg : 512 * (vg + 1)].rearrange(
                        "p (h d) -> p h d", d=64
                    ),
                )

    # ================= phase 2: attention + output projection =================
    with ExitStack() as ph2:
        p2 = ph2.enter_context(tc.tile_pool(name="p2", bufs=1))
        wpool2 = ph2.enter_context(tc.tile_pool(name="wpool2", bufs=2))
        ptp = ph2.enter_context(tc.tile_pool(name="ptp", bufs=6))
        rpool = ph2.enter_context(tc.tile_pool(name="rpool", bufs=2))
        opool = ph2.enter_context(tc.tile_pool(name="opool", bufs=3))
        apsum = ph2.enter_context(tc.tile_pool(name="apsum", bufs=3, space="PSUM"))
        upsum = ph2.enter_context(tc.tile_pool(name="upsum", bufs=2, space="PSUM"))
        bpsum = ph2.enter_context(tc.tile_pool(name="bpsum", bufs=1, space="PSUM"))

        masks = p2.tile([128, 16, QB], BF16, name="masks_sb")
        nc.sync.dma_start(out=masks, in_=io["masks"])
        bOt = p2.tile([128, D], F32, name="bO_sb")
        nc.sync.dma_start(out=bOt, in_=bcast_rows(io["bO"]))
        Ocat = [p2.tile([64, NH, QB], BF16, name=f"ocat{i}") for i in range(4)]

        for i in range(4):
            C = CCH[i]
            for h in range(NH):
                rt, b0 = h // 2, 64 * (h % 2)
                Ups = upsum.tile([65, QB], F32, name="ups", tag="ups")
                for c in range(C):
                    Sps = apsum.tile([128, QB], F32, name="attps", tag="attps")
                    nc.tensor.matmul(
                        Sps,
                        lhsT=Krot[rt][b0 : b0 + 64, 128 * c : 128 * (c + 1)],
                        rhs=Qrot[rt][b0 : b0 + 64, QB * i : QB * (i + 1)],
                        start=True,
                        stop=True,
                    )
                    P = ptp.tile([128, QB], BF16, name="ptile", tag="ptile")
                    nc.scalar.activation(P, Sps, AF.Exp, scale=0.125)
                    w = c - (C - 4)
                    if w >= 0:
                        nc.vector.tensor_mul(P, P, masks[:, 4 * i + w, :])
                    nc.tensor.matmul(
                        Ups,
                        lhsT=Vaug[c][:, h, :],
                        rhs=P,
                        start=(c == 0),
                        stop=(c == C - 1),
                    )
                rc = rpool.tile([65, QB], F32, name="rct", tag="rct")
                nc.vector.reciprocal(rc[64:65, :], Ups[64:65, :])
                Bps = bpsum.tile([64, QB], F32, name="bps", tag="bps")
                nc.tensor.matmul(
                    Bps, lhsT=ones_t[64:65, :], rhs=rc[64:65, :], start=True, stop=True
                )
                # walrus: a TensorTensor may read only one PSUM operand
                bsb = rpool.tile([64, QB], F32, name="bsb", tag="bsb")
                nc.scalar.copy(bsb, Bps)
                nc.vector.tensor_mul(Ocat[i][:, h, :], Ups[0:64, :], bsb)

        # ---- output projection ----
        for eg in range(2):
            woa = wpool2.tile([64, 8, 512], BF16, name=f"woa{eg}", tag="wph2")
            nc.sync.dma_start(
                out=woa, in_=io["woT"][:, 0:8, 512 * eg : 512 * (eg + 1)]
            )
            wob = wpool2.tile([64, 8, 512], BF16, name=f"wob{eg}", tag="wph2")
            nc.sync.dma_start(
                out=wob, in_=io["woT"][:, 8:16, 512 * eg : 512 * (eg + 1)]
            )
            for qt in range(8):
                i, qoff = qt // 2, 128 * (qt % 2)
                ps = ppsum.tile([128, 512], F32, name="ops", tag="projps")
                for ct in range(16):
                    wt = woa if ct < 8 else wob
                    nc.tensor.matmul(
                        ps,
                        lhsT=Ocat[i][:, ct, qoff : qoff + 128],
                        rhs=wt[:, ct % 8, :],
                        start=(ct == 0),
                        stop=(ct == 15),
                    )
                ot = opool.tile([128, 512], F32, name="ot", tag="ot")
                nc.vector.tensor_add(ot, ps, bOt[:, 512 * eg : 512 * (eg + 1)])
                nc.sync.dma_start(
                    out=io["out"][:, qt, 512 * eg : 512 * (eg + 1)], in_=ot
                )


def build_program(debug: bool = False):
    nc = bacc.Bacc("TRN2", target_bir_lowering=False, debug=debug)
    io = {}

    def inp(name, shape, dt):
        io[name] = nc.dram_tensor(name, shape, dt, kind="ExternalInput").ap()

    inp("xT", [128, 8, S], BF16)
    inp("xqT", [128, 8, NQ], BF16)
    inp("wqkvT", [128, 8, 3 * D], BF16)
    inp("woT", [64, 16, D], BF16)
    inp("cosK", [128, S], F16)
    inp("sinK", [128, S], F16)
    inp("cosQ", [128, NQ], F16)
    inp("sinQ", [128, NQ], F16)
    inp("bQK", [128, 16], F32)
    inp("bV", [D], F32)
    inp("bO", [D], F32)
    inp("masks", [128, 16, QB], BF16)
    inp("pswap", [128, 128], BF16)
    io["out"] = nc.dram_tensor("out", [128, 8, D], F32, kind="ExternalOutput").ap()

    from contextlib import ExitStack

    with tile.TileContext(nc) as tc, ExitStack() as ctx:
        emit(tc, io, ctx)
    nc.compile()
    return nc


_PROG = None


def _get_prog():
    global _PROG
    if _PROG is None:
        _PROG = build_program(debug=False)
    return _PROG


def make_in_maps(x, token_positions, W_qkv, b_qkv, W_o, b_o):
    """Host-side sharding/layout prep. Returns (in_maps, per-core (batch, qidx))."""
    x = np.asarray(x, dtype=np.float32)
    pos = np.asarray(token_positions).astype(np.float32)
    W_qkv = np.asarray(W_qkv, dtype=np.float32)
    b_qkv = np.asarray(b_qkv, dtype=np.float32)
    W_o = np.asarray(W_o, dtype=np.float32)
    b_o = np.asarray(b_o, dtype=np.float32)

    wqkvT_r = (
        W_qkv.T.reshape(8, 128, 3 * D).transpose(1, 0, 2).astype(BFNP)
    )  # [128, 8, 3072]; [p, ic, oc] = W_qkv[oc, 128*ic+p]
    woT_r = (
        W_o.T.reshape(16, 64, D).transpose(1, 0, 2).astype(BFNP)
    )  # [64, 16, 1024]; [d, ct, e] = W_o[e, 64*ct+d]

    # RoPE tables in the duplicated-row layout matching Q^T/K^T row tiles:
    # row r <-> head-local channel d = r % 64, freq i = d // 2.
    inv = (10000.0 ** (-np.arange(DK // 2, dtype=np.float32) * 2.0 / DK)).astype(
        np.float32
    )
    r = np.arange(128)
    d_loc = r % 64
    fi = inv[d_loc // 2]  # [128]
    sign = np.where(d_loc % 2 == 0, np.float32(-1.0), np.float32(1.0))

    angK = pos[None, :] * fi[:, None]
    cosK = np.cos(angK).astype(np.float16)
    sinK = (np.sin(angK) * sign[:, None]).astype(np.float16)

    bQK_h = np.zeros((128, 16), np.float32)
    for t in range(8):
        bQK_h[:, t] = b_qkv[128 * t : 128 * (t + 1)]
        bQK_h[:, 8 + t] = b_qkv[D + 128 * t : D + 128 * (t + 1)]
    bV_h = np.ascontiguousarray(b_qkv[2 * D : 3 * D])
    bO_h = np.ascontiguousarray(b_o)

    psw = np.zeros((128, 128), np.float32)
    psw[r, r ^ 1] = 1.0
    psw = psw.astype(BFNP)

    def mk_masks(gl):
        m = np.zeros((128, 16, QB), np.float32)
        pc = np.arange(128)[:, None]
        fc = np.arange(QB)[None, :]
        for i, g in enumerate(gl):
            for w in range(4):
                j = CCH[i] - 4 + w
                m[:, 4 * i + w, :] = (128 * j + pc) <= (QB * g + fc)
        return m.astype(BFNP)

    masks_by_par = [mk_masks(GBLK[0]), mk_masks(GBLK[1])]

    in_maps, meta = [], []
    for core in range(8):
        b, par = core // 2, core % 2
        gl = GBLK[par]
        qidx = np.concatenate([np.arange(QB * g, QB * (g + 1)) for g in gl])
        xb = x[b]
        xT_r = xb.T.reshape(8, 128, S).transpose(1, 0, 2).astype(BFNP)
        xqT_r = xb[qidx].T.reshape(8, 128, NQ).transpose(1, 0, 2).astype(BFNP)
        angQ = pos[qidx][None, :] * fi[:, None]
        cosQ = np.cos(angQ).astype(np.float16)
        sinQ = (np.sin(angQ) * sign[:, None]).astype(np.float16)
        in_maps.append(
            dict(
                xT=xT_r,
                xqT=xqT_r,
                wqkvT=wqkvT_r,
                woT=woT_r,
                cosK=cosK,
                sinK=sinK,
                cosQ=cosQ,
                sinQ=sinQ,
                bQK=bQK_h,
                bV=bV_h,
                bO=bO_h,
                masks=masks_by_par[par],
                pswap=psw,
            )
        )
        meta.append((b, qidx))
    return in_maps, meta


def gather_out(results, meta):
    out = np.empty((4, S, D), np.float32)
    for core, (b, qidx) in enumerate(meta):
        o = np.asarray(results[core]["out"], dtype=np.float32)  # [128, 8, 1024]
        out[b, qidx, :] = o.transpose(1, 0, 2).reshape(NQ, D)
    return out


def kernel(x, token_positions, W_qkv, b_qkv, W_o, b_o):
    from concourse.bass_utils import run_bass_kernel_spmd

    in_maps, meta = make_in_maps(x, token_positions, W_qkv, b_qkv, W_o, b_o)
    res = run_bass_kernel_spmd(_get_prog(), in_maps, list(range(8)))
    return gather_out(res.results, meta)


if __name__ == "__main__":
    nc = build_program(debug=False)
    print("build ok")


# revision 15
# speedup vs baseline: 1.3577x; 1.3577x over previous
"""Trainium2 Bass kernel for fused MHA (QKV proj + RoPE + causal attention + out proj).

Sharding: 8 cores = 4 batches x 2 query-halves. Each core handles one batch's
full K/V (redundant across the pair) and a balanced set of 1024 query rows
(global 256-row blocks {0,3,4,7} or {1,2,5,6}), so causal work is balanced and
all cores run one SPMD program; per-core differences are pure input data
(gathered x columns, RoPE tables at the gathered positions, causal mask tiles).
Host gathers the per-core outputs back into the full [4, 2048, 1024] tensor.

Device algorithm (everything transposed so softmax reduces along PSUM
partitions for free):
  Qb^T = Wq @ xq^T + b      (bf16, PE)
  Qs^T = Pswap @ Qb^T       (pair swap as a matmul; RoPE = Qb*cos + Qs*sin)
  S^T[k,q] = Krot^T(head)^T-slices @ Qrot -> psum, exp(0.125*S) on ACT
  P masked by data-driven causal tiles; U[65,q] += Vaug[k,65]^T P (ones col
  of Vaug accumulates the softmax denominator l)
  O^T = U[0:64] * (1/l broadcast via a K=1 outer-product matmul)
  out[q,e] = sum_heads O^T(head)^T @ Wo^T + b_o
"""

import os
import sys

import numpy as np
import ml_dtypes

for _p in ("/opt/trn_rl_repo",):
    if os.path.isdir(_p) and _p not in sys.path:
        sys.path.insert(0, _p)

import concourse.bass as bass  # noqa: E402
import concourse.tile as tile  # noqa: E402
import concourse.mybir as mybir  # noqa: E402
from concourse import bacc  # noqa: E402

BF16 = mybir.dt.bfloat16
F16 = mybir.dt.float16
F32 = mybir.dt.float32
AF = mybir.ActivationFunctionType
BFNP = ml_dtypes.bfloat16

S = 2048        # sequence length
D = 1024        # model dim
NH = 16         # heads
DK = 64         # head dim
NQ = 1024       # queries per core
QB = 256        # query block (matmul N for attention)
KC = 128        # k chunk (partition tile)
CCH = (4, 8, 12, 16)   # k-chunks computed per local q-block (uniform SPMD)
GBLK = ((0, 3, 4, 7), (1, 2, 5, 6))  # global 256-row blocks by core parity


def emit(tc: tile.TileContext, io: dict, ctx):
    nc = tc.nc
    from contextlib import ExitStack

    # Pools are stack-allocated: phase-1 pools (x, tables, W-staging) are
    # released before phase-2 pools (masks, Ocat, attention temps) allocate,
    # so their SBUF is reclaimed.
    persist = ctx.enter_context(tc.tile_pool(name="persist", bufs=1))

    def bcast_rows(dram_ap):
        # [n] dram vector -> [128, n] partition-broadcast AP
        return bass.AP(
            tensor=dram_ap.tensor,
            offset=dram_ap.offset,
            ap=[[0, 128]] + list(dram_ap.ap),
        )

    # ---- persistent tiles (live across both phases) ----
    bQK = persist.tile([128, 16], F32, name="bQK_sb")
    nc.sync.dma_start(out=bQK, in_=io["bQK"])
    ones_t = persist.tile([65, 64], F16, name="ones_sb")
    nc.vector.memset(ones_t[:, :], 1.0)
    masks = persist.tile([128, 16, QB], BF16, name="masks_sb")
    nc.sync.dma_start(out=masks, in_=io["masks"])

    Qrot = [persist.tile([128, NQ], BF16, name=f"qrot{r}") for r in range(8)]
    Krot = [persist.tile([128, S], BF16, name=f"krot{r}") for r in range(8)]
    Vaug = [persist.tile([128, NH, 65], BF16, name=f"vaug{c}") for c in range(16)]

    for c in range(16):
        nc.vector.memset(Vaug[c][:, :, 64:65], 1.0)

    # ================= phase 1: projections + RoPE =================
    with ExitStack() as ph1:
        p1 = ph1.enter_context(tc.tile_pool(name="p1", bufs=1))
        wpool = ph1.enter_context(tc.tile_pool(name="wpool", bufs=2))
        qkb = ph1.enter_context(tc.tile_pool(name="qkb", bufs=4))
        tmp = ph1.enter_context(tc.tile_pool(name="tmp", bufs=3))
        ppsum = ph1.enter_context(tc.tile_pool(name="ppsum", bufs=2, space="PSUM"))
        spsum = ph1.enter_context(tc.tile_pool(name="spsum", bufs=2, space="PSUM"))

        # DMA order: Q-side inputs first so Q-proj starts ASAP; bulk xT after.
        xqT = p1.tile([128, 8, NQ], BF16, name="xqT_sb")
        nc.sync.dma_start(out=xqT, in_=io["xqT"])
        cosQ = p1.tile([128, NQ], F16, name="cosQ_sb")
        nc.sync.dma_start(out=cosQ, in_=io["cosQ"])
        sinQ = p1.tile([128, NQ], F16, name="sinQ_sb")
        nc.sync.dma_start(out=sinQ, in_=io["sinQ"])
        pswap = p1.tile([128, 128], BF16, name="pswap_sb")
        nc.sync.dma_start(out=pswap, in_=io["pswap"])
        xT = p1.tile([128, 8, S], BF16, name="xT_sb")
        cosK = p1.tile([128, S], F16, name="cosK_sb")
        sinK = p1.tile([128, S], F16, name="sinK_sb")
        bVt = p1.tile([128, D], F32, name="bV_sb")

        def proj_rope(wcol0, rhs, n_sg, cos_t, sin_t, dest, bias0, wname):
            # dest[rt][:, sg] = RoPE( W[:, wcol0+...]^T-slices . rhs + bias )
            for half in range(2):
                wt = wpool.tile([128, 8, 512], BF16, name=f"{wname}{half}", tag="wph")
                nc.sync.dma_start(
                    out=wt,
                    in_=io["wqkvT"][
                        :, :, wcol0 + 512 * half : wcol0 + 512 * (half + 1)
                    ],
                )
                for rtl in range(4):
                    rt = 4 * half + rtl
                    for sg in range(n_sg):
                        ps = ppsum.tile([128, 512], F32, name="projps", tag="projps")
                        for ic in range(8):
                            nc.tensor.matmul(
                                ps,
                                lhsT=wt[:, ic, 128 * rtl : 128 * (rtl + 1)],
                                rhs=rhs[:, ic, 512 * sg : 512 * (sg + 1)],
                                start=(ic == 0),
                                stop=(ic == 7),
                            )
                        qb = qkb.tile([128, 512], BF16, name="qkbt", tag="qkbt")
                        nc.scalar.activation(
                            qb, ps, AF.Identity, bias=bQK[:, bias0 + rt : bias0 + rt + 1]
                        )
                        sps = spsum.tile([128, 512], F32, name="swapps", tag="swapps")
                        nc.tensor.matmul(sps, lhsT=pswap, rhs=qb, start=True, stop=True)
                        nc.vector.tensor_mul(
                            sps, sps, sin_t[:, 512 * sg : 512 * (sg + 1)]
                        )
                        t1 = tmp.tile([128, 512], F32, name="ropet", tag="ropet")
                        nc.vector.tensor_mul(
                            t1, qb, cos_t[:, 512 * sg : 512 * (sg + 1)]
                        )
                        nc.vector.tensor_add(
                            dest[rt][:, 512 * sg : 512 * (sg + 1)], t1, sps
                        )

        proj_rope(0, xqT, 2, cosQ, sinQ, Qrot, 0, "wq")
        nc.sync.dma_start(out=xT, in_=io["xT"])
        nc.sync.dma_start(out=cosK, in_=io["cosK"])
        nc.sync.dma_start(out=sinK, in_=io["sinK"])
        nc.sync.dma_start(out=bVt, in_=bcast_rows(io["bV"]))
        proj_rope(D, xT, 4, cosK, sinK, Krot, 8, "wk")

        # ---- V projection (natural [s, vc] layout) ----
        for vg in range(2):
            wv = wpool.tile([128, 8, 512], BF16, name=f"wv{vg}", tag="wph")
            nc.sync.dma_start(
                out=wv,
                in_=io["wqkvT"][:, :, 2 * D + 512 * vg : 2 * D + 512 * (vg + 1)],
            )
            for st in range(16):
                ps = ppsum.tile([128, 512], F32, name="vps", tag="projps")
                for ic in range(8):
                    nc.tensor.matmul(
                        ps,
                        lhsT=xT[:, ic, 128 * st : 128 * (st + 1)],
                        rhs=wv[:, ic, :],
                        start=(ic == 0),
                        stop=(ic == 7),
                    )
                nc.vector.tensor_add(
                    Vaug[st][:, 8 * vg : 8 * (vg + 1), 0:64],
                    ps.rearrange("p (h d) -> p h d", d=64),
                    bVt[:, 512 * vg : 512 * (vg + 1)].rearrange(
                        "p (h d) -> p h d", d=64
                    ),
                )

    # ================= phase 2: attention + output projection =================
    with ExitStack() as ph2:
        p2 = ph2.enter_context(tc.tile_pool(name="p2", bufs=1))
        wpool2 = ph2.enter_context(tc.tile_pool(name="wpool2", bufs=1))
        ptp = ph2.enter_context(tc.tile_pool(name="ptp", bufs=2))
        rpool = ph2.enter_context(tc.tile_pool(name="rpool", bufs=2))
        opool = ph2.enter_context(tc.tile_pool(name="opool", bufs=3))
        apsum = ph2.enter_context(tc.tile_pool(name="apsum", bufs=1, space="PSUM"))
        upsum = ph2.enter_context(tc.tile_pool(name="upsum", bufs=1, space="PSUM"))
        bpsum = ph2.enter_context(tc.tile_pool(name="bpsum", bufs=1, space="PSUM"))

        bOt = p2.tile([128, D], F32, name="bO_sb")
        nc.sync.dma_start(out=bOt, in_=bcast_rows(io["bO"]))
        Ocat = [p2.tile([64, NH, QB], BF16, name=f"ocat{i}") for i in range(4)]
        wo = []
        for eg in range(2):
            woa = wpool2.tile([64, 8, 512], BF16, name=f"woa{eg}", tag=f"woa{eg}")
            nc.sync.dma_start(
                out=woa, in_=io["woT"][:, 0:8, 512 * eg : 512 * (eg + 1)]
            )
            wob = wpool2.tile([64, 8, 512], BF16, name=f"wob{eg}", tag=f"wob{eg}")
            nc.sync.dma_start(
                out=wob, in_=io["woT"][:, 8:16, 512 * eg : 512 * (eg + 1)]
            )
            wo.append((woa, wob))

        for i in range(4):
            C = CCH[i]
            G = C // 4
            for hp in range(8):
                rt = hp
                heads = (2 * hp, 2 * hp + 1)
                Ups = [
                    upsum.tile([65, QB], F32, name=f"ups{j}", tag=f"ups{j}")
                    for j in range(2)
                ]
                for g in range(G):
                    Sp = [
                        apsum.tile([128, 4 * QB], F32, name=f"attps{j}", tag=f"attps{j}")
                        for j in range(2)
                    ]
                    # even/odd heads hit disjoint PE row-quadrants (lhsT base
                    # partition 0 vs 64) -> adjacent QKs can overlap in the array
                    for cc in range(4):
                        c = 4 * g + cc
                        for j in range(2):
                            b0 = 64 * j
                            nc.tensor.matmul(
                                Sp[j][:, QB * cc : QB * (cc + 1)],
                                lhsT=Krot[rt][b0 : b0 + 64, 128 * c : 128 * (c + 1)],
                                rhs=Qrot[rt][b0 : b0 + 64, QB * i : QB * (i + 1)],
                                start=True,
                                stop=True,
                            )
                    Pk = [
                        ptp.tile([128, 4 * QB], BF16, name=f"ptile{j}", tag=f"ptile{j}")
                        for j in range(2)
                    ]
                    for j in range(2):
                        nc.scalar.activation(Pk[j], Sp[j], AF.Exp, scale=0.125)
                    if g == G - 1:
                        mv = masks[:, 4 * i : 4 * (i + 1), :]
                        for j in range(2):
                            Pv = Pk[j].rearrange("p (c q) -> p c q", q=QB)
                            nc.vector.tensor_mul(Pv, Pv, mv)
                    for cc in range(4):
                        c = 4 * g + cc
                        for j in range(2):
                            nc.tensor.matmul(
                                Ups[j],
                                lhsT=Vaug[c][:, heads[j], :],
                                rhs=Pk[j][:, QB * cc : QB * (cc + 1)],
                                start=(c == 0),
                                stop=(c == C - 1),
                            )
                for j in range(2):
                    rc = rpool.tile([65, QB], F16, name="rct", tag="rct")
                    with nc.allow_low_precision(reason="1/l in fp16 feeds fp16 bcast matmul"):
                        nc.vector.reciprocal(rc[64:65, :], Ups[j][64:65, :])
                    Bps = bpsum.tile([64, QB], F32, name="bps", tag="bps")
                    nc.tensor.matmul(
                        Bps,
                        lhsT=ones_t[64:65, :],
                        rhs=rc[64:65, :],
                        start=True,
                        stop=True,
                    )
                    # walrus: a TensorTensor may read only one PSUM operand
                    bsb = rpool.tile([64, QB], F32, name="bsb", tag="bsb")
                    nc.scalar.copy(bsb, Bps)
                    nc.vector.tensor_mul(
                        Ocat[i][:, heads[j], :], Ups[j][0:64, :], bsb
                    )
            # ---- output projection for this block (PE filler during next block) ----
            for qt in (2 * i, 2 * i + 1):
                qoff = 128 * (qt % 2)
                for eg in range(2):
                    woa, wob = wo[eg]
                    ps = bpsum.tile([128, 512], F32, name="ops", tag="ops")
                    for ct in range(16):
                        wt = woa if ct < 8 else wob
                        nc.tensor.matmul(
                            ps,
                            lhsT=Ocat[i][:, ct, qoff : qoff + 128],
                            rhs=wt[:, ct % 8, :],
                            start=(ct == 0),
                            stop=(ct == 15),
                        )
                    ot = opool.tile([128, 512], F32, name="ot", tag="ot")
                    nc.vector.tensor_add(ot, ps, bOt[:, 512 * eg : 512 * (eg + 1)])
                    nc.sync.dma_start(
                        out=io["out"][:, qt, 512 * eg : 512 * (eg + 1)], in_=ot
                    )


def build_program(debug: bool = False):
    nc = bacc.Bacc("TRN2", target_bir_lowering=False, debug=debug)
    io = {}

    def inp(name, shape, dt):
        io[name] = nc.dram_tensor(name, shape, dt, kind="ExternalInput").ap()

    inp("xT", [128, 8, S], BF16)
    inp("xqT", [128, 8, NQ], BF16)
    inp("wqkvT", [128, 8, 3 * D], BF16)
    inp("woT", [64, 16, D], BF16)
    inp("cosK", [128, S], F16)
    inp("sinK", [128, S], F16)
    inp("cosQ", [128, NQ], F16)
    inp("sinQ", [128, NQ], F16)
    inp("bQK", [128, 16], F32)
    inp("bV", [D], F32)
    inp("bO", [D], F32)
    inp("masks", [128, 16, QB], BF16)
    inp("pswap", [128, 128], BF16)
    io["out"] = nc.dram_tensor("out", [128, 8, D], F32, kind="ExternalOutput").ap()

    from contextlib import ExitStack

    with tile.TileContext(nc) as tc, ExitStack() as ctx:
        emit(tc, io, ctx)
    nc.compile()
    return nc


_PROG = None


def _get_prog():
    global _PROG
    if _PROG is None:
        _PROG = build_program(debug=False)
    return _PROG


def make_in_maps(x, token_positions, W_qkv, b_qkv, W_o, b_o):
    """Host-side sharding/layout prep. Returns (in_maps, per-core (batch, qidx))."""
    x = np.asarray(x, dtype=np.float32)
    pos = np.asarray(token_positions).astype(np.float32)
    W_qkv = np.asarray(W_qkv, dtype=np.float32)
    b_qkv = np.asarray(b_qkv, dtype=np.float32)
    W_o = np.asarray(W_o, dtype=np.float32)
    b_o = np.asarray(b_o, dtype=np.float32)

    wqkvT_r = (
        W_qkv.T.reshape(8, 128, 3 * D).transpose(1, 0, 2).astype(BFNP)
    )  # [128, 8, 3072]; [p, ic, oc] = W_qkv[oc, 128*ic+p]
    woT_r = (
        W_o.T.reshape(16, 64, D).transpose(1, 0, 2).astype(BFNP)
    )  # [64, 16, 1024]; [d, ct, e] = W_o[e, 64*ct+d]

    # RoPE tables in the duplicated-row layout matching Q^T/K^T row tiles:
    # row r <-> head-local channel d = r % 64, freq i = d // 2.
    inv = (10000.0 ** (-np.arange(DK // 2, dtype=np.float32) * 2.0 / DK)).astype(
        np.float32
    )
    r = np.arange(128)
    d_loc = r % 64
    fi = inv[d_loc // 2]  # [128]
    sign = np.where(d_loc % 2 == 0, np.float32(-1.0), np.float32(1.0))

    angK = pos[None, :] * fi[:, None]
    cosK = np.cos(angK).astype(np.float16)
    sinK = (np.sin(angK) * sign[:, None]).astype(np.float16)

    bQK_h = np.zeros((128, 16), np.float32)
    for t in range(8):
        bQK_h[:, t] = b_qkv[128 * t : 128 * (t + 1)]
        bQK_h[:, 8 + t] = b_qkv[D + 128 * t : D + 128 * (t + 1)]
    bV_h = np.ascontiguousarray(b_qkv[2 * D : 3 * D])
    bO_h = np.ascontiguousarray(b_o)

    psw = np.zeros((128, 128), np.float32)
    psw[r, r ^ 1] = 1.0
    psw = psw.astype(BFNP)

    def mk_masks(gl):
        m = np.zeros((128, 16, QB), np.float32)
        pc = np.arange(128)[:, None]
        fc = np.arange(QB)[None, :]
        for i, g in enumerate(gl):
            for w in range(4):
                j = CCH[i] - 4 + w
                m[:, 4 * i + w, :] = (128 * j + pc) <= (QB * g + fc)
        return m.astype(BFNP)

    masks_by_par = [mk_masks(GBLK[0]), mk_masks(GBLK[1])]

    in_maps, meta = [], []
    for core in range(8):
        b, par = core // 2, core % 2
        gl = GBLK[par]
        qidx = np.concatenate([np.arange(QB * g, QB * (g + 1)) for g in gl])
        xb = x[b]
        xT_r = xb.T.reshape(8, 128, S).transpose(1, 0, 2).astype(BFNP)
        xqT_r = xb[qidx].T.reshape(8, 128, NQ).transpose(1, 0, 2).astype(BFNP)
        angQ = pos[qidx][None, :] * fi[:, None]
        cosQ = np.cos(angQ).astype(np.float16)
        sinQ = (np.sin(angQ) * sign[:, None]).astype(np.float16)
        in_maps.append(
            dict(
                xT=xT_r,
                xqT=xqT_r,
                wqkvT=wqkvT_r,
                woT=woT_r,
                cosK=cosK,
                sinK=sinK,
                cosQ=cosQ,
                sinQ=sinQ,
                bQK=bQK_h,
                bV=bV_h,
                bO=bO_h,
                masks=masks_by_par[par],
                pswap=psw,
            )
        )
        meta.append((b, qidx))
    return in_maps, meta


def gather_out(results, meta):
    out = np.empty((4, S, D), np.float32)
    for core, (b, qidx) in enumerate(meta):
        o = np.asarray(results[core]["out"], dtype=np.float32)  # [128, 8, 1024]
        out[b, qidx, :] = o.transpose(1, 0, 2).reshape(NQ, D)
    return out


def kernel(x, token_positions, W_qkv, b_qkv, W_o, b_o):
    from concourse.bass_utils import run_bass_kernel_spmd

    in_maps, meta = make_in_maps(x, token_positions, W_qkv, b_qkv, W_o, b_o)
    res = run_bass_kernel_spmd(_get_prog(), in_maps, list(range(8)))
    return gather_out(res.results, meta)


if __name__ == "__main__":
    nc = build_program(debug=False)
    print("build ok")
